# revision 2
# baseline (speedup 1.0000x reference)
"""Causal single-head attention (B=4, S=2048, D=1024) on 8 trn2 NeuronCores.

Scheme B: algebraic refactor that never materializes K or V.
  scores = (Xq @ W2) @ Xk^T + brow @ Xk^T      W2 = Wq.T @ Wk / sqrt(D)
                                               brow = bq @ Wk / sqrt(D)
  (bk contributes a per-row constant to scores -> cancels in softmax)
  out    = (P @ Xk) @ Wv.T / den + bv          (P rows sum to den)

Per-core matmul work drops from 8.05e9 MACs (baseline: Q/K/V projections
with K/V duplicated across the 2 cores of a batch) to 4.82e9:
  Q' = Xq@W2 (1.07e9) + scores (1.34e9) + Z = P@Xk (1.34e9) + Z@Wv.T (1.07e9).

Sharding: 8 cores = 4 batches x 2 query-interleaved shards, as baseline.
Core (b, p) handles 256-row query chunks p, p+2, p+4, p+6 of batch b.

Layouts (host pre-transposed, no on-chip transposes):
  Q'^T[e, q]   = W2(stationary, natural [d,e]) @ Xq^T
  scores^T[k,q]= Xk^T-tile(stationary) @ Q'^T    (contract e)
  Z^T[d, q]    = Xk-natural-tile(stationary) @ p (contract k)
  O[q, e]      = Z^T-slice(stationary) @ Wv.T-natural (contract d)
"""

import numpy as np
import ml_dtypes
from contextlib import ExitStack

import concourse.bacc as bacc
import concourse.bass as bass
import concourse.mybir as mybir
import concourse.tile as tile
from concourse import bass_utils

bf16 = ml_dtypes.bfloat16
f32 = np.float32

B, S, D = 4, 2048, 1024
E = D
N_CORES = 8
QCH = 256          # query chunk rows (per-core local chunk)
NCH = 4            # local chunks per core
SQ = QCH * NCH     # 1024 query rows per core
DT = D // 128      # 8 d-tiles
ET = E // 128      # 8 e-tiles
KT = S // 128      # 16 key tiles

_CACHE = {}


def _build(reps=1):
    nc = bacc.Bacc("TRN2")
    dt_bf16 = mybir.dt.bfloat16
    dt_f32 = mybir.dt.float32

    xtq = nc.dram_tensor("xtq", [D, SQ], dt_bf16, kind="ExternalInput")
    xtkv = nc.dram_tensor("xtkv", [D, S], dt_bf16, kind="ExternalInput")
    xnat = nc.dram_tensor("xnat", [S, D], dt_bf16, kind="ExternalInput")
    wpp = nc.dram_tensor("wpp", [D, E], dt_bf16, kind="ExternalInput")
    wvn = nc.dram_tensor("wvn", [D, E], dt_bf16, kind="ExternalInput")
    brows = nc.dram_tensor("brows", [128, ET], dt_f32, kind="ExternalInput")
    bvv = nc.dram_tensor("bvv", [1, E], dt_f32, kind="ExternalInput")
    maskt = nc.dram_tensor("maskt", [512, QCH], dt_bf16, kind="ExternalInput")
    o = nc.dram_tensor("o", [SQ, E], dt_f32, kind="ExternalOutput")

    Ident = mybir.ActivationFunctionType.Identity
    Exp = mybir.ActivationFunctionType.Exp

    with ExitStack() as ctx:
        tc = ctx.enter_context(tile.TileContext(nc))
        persist = ctx.enter_context(tc.tile_pool(name="persist", bufs=1))

        qpt = [persist.tile([128, SQ], dt_bf16, tag=f"qpt{i}", name=f"qpt{i}") for i in range(ET)]
        xkv = [persist.tile([128, S], dt_bf16, tag=f"xkv{i}", name=f"xkv{i}") for i in range(ET)]
        xn = [persist.tile([128, D], dt_bf16, tag=f"xn{i}", name=f"xn{i}") for i in range(KT)]
        wv = [persist.tile([128, E], dt_bf16, tag=f"wv{i}", name=f"wv{i}") for i in range(DT)]
        msk = [persist.tile([128, QCH], dt_bf16, tag=f"m{i}", name=f"m{i}") for i in range(4)]
        brow_sb = persist.tile([128, ET], dt_f32, tag="brow")
        bv_bc = persist.tile([128, E], dt_f32, tag="bvbc")
        ones_col = persist.tile([128, 1], dt_bf16, tag="ones")

        nc.vector.memset(ones_col[:], 1.0)

        for _rep in range(reps):
            # ---------------- Phase 1: Q' = Xq @ W2 (+ brow) ----------------
            with (
                tc.tile_pool(name="p1", bufs=1) as p1,
                tc.tile_pool(name="psp1", bufs=8, space="PSUM") as psp1,
            ):
                xq = [p1.tile([128, SQ], dt_bf16, tag=f"xq{i}", name=f"xq{i}") for i in range(DT)]
                wp = [p1.tile([128, E], dt_bf16, tag=f"wp{i}", name=f"wp{i}") for i in range(DT)]

                # warm-up matmuls trip the HAM clock gate to 2.4GHz during
                # the DMA lead-in (see baseline)
                warm = p1.tile([128, 512], dt_bf16, tag="warm", name="warm")
                nc.vector.memset(warm[:], 0.0)
                wps = psp1.tile([128, 512], dt_f32, tag="ps", name="pswarm")
                for _ in range(12):
                    nc.tensor.matmul(wps[:], warm[:, 0:128], warm[:],
                                     start=True, stop=True)

                # dt=0 tiles in column halves so the first matmul starts early
                for h in range(2):
                    nc.sync.dma_start(out=wp[0][:, h * 512:(h + 1) * 512],
                                      in_=wpp.ap()[0:128, h * 512:(h + 1) * 512])
                    nc.sync.dma_start(out=xq[0][:, h * 512:(h + 1) * 512],
                                      in_=xtq.ap()[0:128, h * 512:(h + 1) * 512])
                if _rep == 0:
                    nc.sync.dma_start(out=brow_sb[:], in_=brows.ap())
                for i in range(1, DT):
                    nc.sync.dma_start(out=wp[i][:], in_=wpp.ap()[i * 128:(i + 1) * 128, :])
                    nc.sync.dma_start(out=xq[i][:], in_=xtq.ap()[i * 128:(i + 1) * 128, :])
                if _rep == 0:
                    for i in range(4):
                        nc.sync.dma_start(out=msk[i][:],
                                          in_=maskt.ap()[i * 128:(i + 1) * 128, :])
                # phase-2 operands, in first-use order: xkv columns 0:1024
                # (chunks 0-1), xn tiles 0-7, wv, xkv columns 1024:2048,
                # xn tiles 8-15, bv
                for i in range(ET):
                    nc.sync.dma_start(out=xkv[i][:, 0:1024],
                                      in_=xtkv.ap()[i * 128:(i + 1) * 128, 0:1024])
                for i in range(8):
                    nc.sync.dma_start(out=xn[i][:], in_=xnat.ap()[i * 128:(i + 1) * 128, :])
                for i in range(DT):
                    nc.sync.dma_start(out=wv[i][:], in_=wvn.ap()[i * 128:(i + 1) * 128, :])
                for i in range(ET):
                    nc.sync.dma_start(out=xkv[i][:, 1024:2048],
                                      in_=xtkv.ap()[i * 128:(i + 1) * 128, 1024:2048])
                for i in range(8, KT):
                    nc.sync.dma_start(out=xn[i][:], in_=xnat.ap()[i * 128:(i + 1) * 128, :])
                if _rep == 0:
                    bv_ap = bass.AP(tensor=bvv, offset=0, ap=[[0, 128], [1, E]])
                    nc.gpsimd.dma_start(out=bv_bc[:], in_=bv_ap)

                # Q'^T[e, sq] = W2^T.T @ Xq^T, with brow as per-partition bias
                for ep in range(0, ET, 2):
                    grp = [(et, c) for et in (ep, ep + 1) for c in range(SQ // 512)]
                    pss_ = [psp1.tile([128, 512], dt_f32, tag="ps", name=f"psq{ep}_{gi}")
                            for gi in range(len(grp))]
                    for dt in range(DT):
                        for gi, (et, c) in enumerate(grp):
                            nc.tensor.matmul(
                                pss_[gi][:],
                                wp[dt][:, et * 128:(et + 1) * 128],
                                xq[dt][:, c * 512:(c + 1) * 512],
                                start=(dt == 0), stop=(dt == DT - 1),
                            )
                    for gi, (et, c) in enumerate(grp):
                        nc.scalar.activation(
                            qpt[et][:, c * 512:(c + 1) * 512], pss_[gi][:], Ident,
                            bias=brow_sb[:, et:et + 1],
                        )

            # ---------------- Phase 2: attention ----------------
            with (
                tc.tile_pool(name="p2", bufs=1) as p2,
                tc.tile_pool(name="pss", bufs=2, space="PSUM") as pss,
                tc.tile_pool(name="psz", bufs=4, space="PSUM") as psz,
                tc.tile_pool(name="pso", bufs=2, space="PSUM") as pso,
            ):
                for c in range(NCH):
                    nkt = 4 * (c + 1)
                    qc = c * QCH
                    # scores^T[k, q] then p = exp
                    pts = []
                    for kt in range(nkt):
                        sps = pss.tile([128, QCH], dt_f32, tag="st")
                        for et in range(ET):
                            nc.tensor.matmul(
                                sps[:],
                                xkv[et][:, kt * 128:(kt + 1) * 128],
                                qpt[et][:, qc:qc + QCH],
                                start=(et == 0), stop=(et == ET - 1),
                            )
                        pt = p2.tile([128, QCH], dt_bf16, tag="pt", bufs=32)
                        nc.scalar.activation(pt[:], sps[:], Exp)
                        if kt >= nkt - 4:
                            nc.vector.tensor_mul(pt[:], pt[:], msk[kt - (nkt - 4)][:])
                        pts.append(pt)

                    # softmax denominator: den[q] = sum_k p[k, q]
                    dhr = []
                    for h in range(2):
                        dps = pss.tile([128, QCH], dt_f32, tag="st",
                                       name=f"dps{c}_{h}")
                        hs = slice(h * 128, (h + 1) * 128)
                        for kt in range(nkt):
                            nc.tensor.matmul(dps[:, 0:1], pts[kt][:, hs],
                                             ones_col[:],
                                             start=(kt == 0), stop=(kt == nkt - 1))
                        den_r = p2.tile([128, 1], dt_f32, tag="denr", bufs=4)
                        nc.vector.reciprocal(den_r[:], dps[:, 0:1])
                        dhr.append(den_r)

                    # Z^T[d, q] = Xk-nat^T.T @ p, in 2 passes of 4 d-tiles
                    zt = []
                    for dpass in range(2):
                        zps_ = [psz.tile([128, QCH], dt_f32, tag="z",
                                         name=f"zps{c}_{dpass}_{j}")
                                for j in range(4)]
                        for kt in range(nkt):
                            for j in range(4):
                                dt = dpass * 4 + j
                                nc.tensor.matmul(
                                    zps_[j][:],
                                    xn[kt][:, dt * 128:(dt + 1) * 128],
                                    pts[kt][:],
                                    start=(kt == 0), stop=(kt == nkt - 1),
                                )
                        for j in range(4):
                            zs = p2.tile([128, QCH], dt_bf16, tag="zt", bufs=16)
                            nc.scalar.activation(zs[:], zps_[j][:], Ident)
                            zt.append(zs)

                    # O[q, e] = Z^T.T @ Wv.T ; normalize + bv ; store
                    for h in range(2):
                        hs = slice(h * 128, (h + 1) * 128)
                        o_sb = p2.tile([128, E], dt_f32, tag="osb", bufs=2)
                        for eh in range(2):
                            es = slice(eh * 512, (eh + 1) * 512)
                            ops = pso.tile([128, 512], dt_f32, tag="o")
                            for dt in range(DT):
                                nc.tensor.matmul(
                                    ops[:], zt[dt][:, hs], wv[dt][:, es],
                                    start=(dt == 0), stop=(dt == DT - 1),
                                )
                            nc.vector.tensor_scalar_mul(o_sb[:, es], ops[:], dhr[h][:])
                            nc.vector.tensor_add(o_sb[:, es], o_sb[:, es],
                                                 bv_bc[:, es])
                            nc.sync.dma_start(
                                out=o.ap()[qc + h * 128: qc + (h + 1) * 128, es],
                                in_=o_sb[:, es],
                            )

    nc.compile()
    return nc


def _host_shard(inputs, Wq, bq, Wk, bk, Wv, bv):
    """Build the 8 per-core input maps."""
    scale = np.sqrt(np.float32(D))
    W2 = (Wq.T.astype(f32) @ Wk.astype(f32)) / scale          # [D, D]
    brow = (bq.astype(f32) @ Wk.astype(f32)) / scale          # [D]
    wpp = np.ascontiguousarray(W2).astype(bf16)
    wvn = np.ascontiguousarray(Wv.T).astype(bf16)
    brows = np.ascontiguousarray(brow.reshape(ET, 128).T).astype(f32)
    bvv = np.ascontiguousarray(bv.reshape(1, E)).astype(f32)

    # masks: [512 keys, 256 q], multiplicative (identical to baseline)
    kk = np.arange(512)[:, None]
    qq = np.arange(QCH)[None, :]
    mask_p0 = np.where(kk < 256, (kk <= qq), False).astype(bf16)
    mask_p1 = np.where(kk < 256, True, (kk - 256) <= qq).astype(bf16)
    masks = [mask_p0, mask_p1]

    in_maps = []
    for core in range(N_CORES):
        b, p = divmod(core, 2)
        xb = inputs[b]                       # [S, D] fp32
        rows = np.concatenate(
            [xb[QCH * (2 * c + p): QCH * (2 * c + p) + QCH] for c in range(NCH)],
            axis=0,
        )                                    # [SQ, D]
        in_maps.append({
            "xtq": np.ascontiguousarray(rows.T).astype(bf16),
            "xtkv": np.ascontiguousarray(xb.T).astype(bf16),
            "xnat": np.ascontiguousarray(xb).astype(bf16),
            "wpp": wpp, "wvn": wvn,
            "brows": brows, "bvv": bvv,
            "maskt": masks[p],
        })
    return in_maps


def _assemble(results, dtype):
    out = np.empty((B, S, E), dtype=dtype)
    for core in range(N_CORES):
        b, p = divmod(core, 2)
        oc = results[core]["o"]
        for c in range(NCH):
            g = 2 * c + p
            out[b, QCH * g: QCH * (g + 1)] = oc[QCH * c: QCH * (c + 1)]
    return out


def kernel(inputs, Wq, bq, Wk, bk, Wv, bv):
    inputs = np.asarray(inputs, dtype=f32)
    Wq, bq = np.asarray(Wq, dtype=f32), np.asarray(bq, dtype=f32)
    Wk, bk = np.asarray(Wk, dtype=f32), np.asarray(bk, dtype=f32)
    Wv, bv = np.asarray(Wv, dtype=f32), np.asarray(bv, dtype=f32)

    if "nc" not in _CACHE:
        _CACHE["nc"] = _build()
    nc = _CACHE["nc"]

    in_maps = _host_shard(inputs, Wq, bq, Wk, bk, Wv, bv)
    res = bass_utils.run_bass_kernel_spmd(nc, in_maps, core_ids=list(range(N_CORES)))
    return _assemble(res.results, f32)
